# revision 50
# baseline (speedup 1.0000x reference)
"""EMAttention2d (vq_codebook) Trainium2 kernel, v2.

Data parallel over batch: 16 images -> 8 cores x 2 images. BN batch stats
cross-core reduced with a tiny AllReduce.

Key layout change vs v1: the EM loop works in pixel-partition layout so
softmax needs no transposes, and all big matmuls keep a full 128-row
output partition with bf16 moving operands (1 cycle/row on PE):

  per image (X = x[b] as (C,N), host also supplies X^T; both bf16):
    mu~ = Ws^T mu          (C,K)  [stem folded into codebook; host for it0]
    mub = mu^T bs          (K,)   [host for it0]
    repeat 3x:
      A[n,k]  = X^T mu~ + 1 (x) mub    - 4 chunk matmuls + rank-1 per tile
      E       = exp(A)                 (N,K) bf16, pixel-partition
      z       = E / rowsum(E)          softmax over free dim k
      Gx[c,k] = sum_n X[c,n] z[n,k]    - lhsT = X^T tiles, rhs = z tiles
      sk      = 1^T z
      muR     = Ws Gx + bs (x) sk      (C,K) natural layout
      mu      = muR / ||col||_2        - norm via ones^T muR^2 matmuls
    y2   = mu z^T   (relu) -> head Hw  - z^T via 1c/r bf16 PE transposes
  BN over batch (AllReduce of per-channel sum/sumsq), then
    out = relu(h*a + b2 + x),  a = gamma*rstd, b2 = beta - mean*a

x stays resident in SBUF (bf16) so the final pass reloads nothing.
"""

import sys

for _p in ("/opt/trn_rl_repo",):
    if _p not in sys.path:
        sys.path.insert(0, _p)

import numpy as np

B, C, N, K = 16, 512, 4096, 64
NCORES = 8
BPC = B // NCORES  # images per core
P = 128
OC = C // P   # 4 chunks of channels
NT = N // P   # 32 pixel tiles
NBK = 4       # A-banks per EM iteration
TPB = NT // NBK  # pixel tiles per bank (8)
FCH = N // 1024  # final-pass chunks per (img, o2)
BN_EPS = 1e-5
NUM_ITER = 3

_cache = {}


def _build_nc(n_devices=NCORES, use_collective=True, debug_dumps=False):
    import concourse.bass as bass
    import concourse.mybir as mybir
    import concourse.tile as tile
    from concourse.masks import make_identity
    from contextlib import ExitStack

    dt = mybir.dt
    f32 = dt.float32
    bf16 = dt.float16  # fp16 storage: 8x finer rounding than bf16, same engine rates
    bfr = dt.bfloat16
    AF = mybir.ActivationFunctionType
    ALU = mybir.AluOpType
    AX = mybir.AxisListType

    nc = bass.Bass("TRN2", target_bir_lowering=False, debug=False,
                   num_devices=n_devices)

    xb_d = nc.dram_tensor("xb", [BPC, P, OC, N], bf16, kind="ExternalInput").ap()
    xt_d = nc.dram_tensor("xt", [BPC, P, NT, C], bf16, kind="ExternalInput").ap()
    ws_d = nc.dram_tensor("ws", [P, OC, C], bf16, kind="ExternalInput").ap()
    wst_d = nc.dram_tensor("wst", [P, OC, C], bf16, kind="ExternalInput").ap()
    hwt_d = nc.dram_tensor("hwt", [P, OC, C], bf16, kind="ExternalInput").ap()
    mut0_d = nc.dram_tensor("mut0", [P, OC, K], bf16, kind="ExternalInput").ap()
    mub0_d = nc.dram_tensor("mub0", [1, K], bf16, kind="ExternalInput").ap()
    bsr_d = nc.dram_tensor("bsr", [1, C], bf16, kind="ExternalInput").ap()
    bsc_d = nc.dram_tensor("bsc", [P, OC], bf16, kind="ExternalInput").ap()
    onec_d = nc.dram_tensor("onec", [P, 1], bf16, kind="ExternalInput").ap()
    oner_d = nc.dram_tensor("oner", [1, P], bf16, kind="ExternalInput").ap()
    gm_d = nc.dram_tensor("gm", [P, OC], f32, kind="ExternalInput").ap()
    bt_d = nc.dram_tensor("bt", [P, OC], f32, kind="ExternalInput").ap()
    out_d = nc.dram_tensor("out", [BPC, C, N], bf16, kind="ExternalOutput").ap()
    st_in_d = nc.dram_tensor("stats_in", [P, 2 * OC], f32).ap()
    st_out_d = nc.dram_tensor("stats_out", [P, 2 * OC], f32,
                              addr_space="Shared").ap()
    if debug_dumps:
        bf16_ = dt.float16
        dbg = {
            "dbg_id": nc.dram_tensor("dbg_id", [P, P], bf16_,
                                     kind="ExternalOutput").ap(),
            "dbg_z": nc.dram_tensor("dbg_z", [P, NT, K], bf16_,
                                    kind="ExternalOutput").ap(),
            "dbg_mut": nc.dram_tensor("dbg_mut", [P, OC, K], bf16_,
                                      kind="ExternalOutput").ap(),
            "dbg_muT": nc.dram_tensor("dbg_muT", [K, C], bf16_,
                                      kind="ExternalOutput").ap(),
            "dbg_rs": nc.dram_tensor("dbg_rs", [K, 1], f32,
                                     kind="ExternalOutput").ap(),
            "dbg_h": nc.dram_tensor("dbg_h", [P, OC, N], bf16_,
                                    kind="ExternalOutput").ap(),
            "dbg_pack": nc.dram_tensor("dbg_pack", [P, 2 * OC], f32,
                                       kind="ExternalOutput").ap(),
            "dbg_ab": nc.dram_tensor("dbg_ab", [P, 2 * OC], f32,
                                     kind="ExternalOutput").ap(),
            "dbg_xb": nc.dram_tensor("dbg_xb", [P, OC, 1024], bf16_,
                                     kind="ExternalOutput").ap(),
            "dbg_xt": nc.dram_tensor("dbg_xt", [P, NT, C], bf16_,
                                     kind="ExternalOutput").ap(),
            "dbg_et": nc.dram_tensor("dbg_et", [P, 512], f32,
                                     kind="ExternalOutput").ap(),
            "dbg_mut0": nc.dram_tensor("dbg_mut0", [P, OC, K], bf16_,
                                       kind="ExternalOutput").ap(),
            "dbg_gx": nc.dram_tensor("dbg_gx", [P, OC, K], bf16_,
                                     kind="ExternalOutput").ap(),
            "dbg_sk": nc.dram_tensor("dbg_sk", [1, K], bf16_,
                                     kind="ExternalOutput").ap(),
            "dbg_rsr": nc.dram_tensor("dbg_rsr", [1, K], bf16_,
                                      kind="ExternalOutput").ap(),
            "dbg_mun": nc.dram_tensor("dbg_mun", [P, OC, K], bf16_,
                                      kind="ExternalOutput").ap(),
            "dbg_nsq": nc.dram_tensor("dbg_nsq", [1, K], f32,
                                      kind="ExternalOutput").ap(),
        }

    with tile.TileContext(nc) as tc, ExitStack() as ctx:
        consts = ctx.enter_context(tc.tile_pool(name="consts", bufs=1))
        xbig = ctx.enter_context(tc.tile_pool(name="xbig", bufs=1))
        zpool = ctx.enter_context(tc.tile_pool(name="zpool", bufs=1))
        mutp = ctx.enter_context(tc.tile_pool(name="mutp", bufs=2))
        smalls = ctx.enter_context(tc.tile_pool(name="smalls", bufs=2))
        statp = ctx.enter_context(tc.tile_pool(name="statp", bufs=1))

        # ---- constants ----
        idb = consts.tile([P, P], bf16)
        make_identity(nc, idb[:])
        if debug_dumps:
            nc.sync.dma_start(dbg["dbg_id"], idb[:])
        ws_sb = consts.tile([P, OC, C], bf16)
        nc.sync.dma_start(ws_sb[:], ws_d)
        wst_sb = consts.tile([P, OC, C], bf16)
        nc.sync.dma_start(wst_sb[:], wst_d)
        mut0_sb = consts.tile([P, OC, K], bf16)
        nc.sync.dma_start(mut0_sb[:], mut0_d)
        mub0_sb = consts.tile([1, K], bf16)
        nc.sync.dma_start(mub0_sb[:], mub0_d)
        bsr_sb = consts.tile([1, C], bf16)
        nc.sync.dma_start(bsr_sb[:], bsr_d)
        bsc_sb = consts.tile([P, OC], bf16)
        nc.sync.dma_start(bsc_sb[:], bsc_d)
        onec_sb = consts.tile([P, 1], bf16)
        nc.sync.dma_start(onec_sb[:], onec_d)
        oner_sb = consts.tile([1, P], bf16)
        nc.sync.dma_start(oner_sb[:], oner_d)
        gm_sb = consts.tile([P, OC], f32)
        nc.sync.dma_start(gm_sb[:], gm_d)
        bt_sb = consts.tile([P, OC], f32)
        nc.sync.dma_start(bt_sb[:], bt_d)
        eps_sb = consts.tile([P, 1], f32)
        nc.vector.memset(eps_sb[:], BN_EPS)

        xb_sb = [xbig.tile([P, OC, N], bf16, name=f"xb{b}") for b in range(BPC)]
        z_sb = [zpool.tile([P, NT, K], bf16, name=f"z{b}") for b in range(BPC)]

        sum_acc = statp.tile([P, OC, BPC * (N // 512)], f32)
        sq_acc = statp.tile([P, OC, BPC * (N // 1024)], f32)

        mut_cur = [mut0_sb, mut0_sb]
        mub_cur = [mub0_sb, mub0_sb]
        rs_col = [None] * BPC
        muT_sb = [None] * BPC

        # ================= EM phase (both images, interleaved) ============
        with ExitStack() as l2:
            xtp = l2.enter_context(tc.tile_pool(name="xtp", bufs=1))
            etp = l2.enter_context(tc.tile_pool(name="etp", bufs=4))
            psum2 = l2.enter_context(tc.tile_pool(name="psum2", bufs=1,
                                                  space="PSUM"))

            xt_sb = [xtp.tile([P, NT, C], bf16, name=f"xt{b}")
                     for b in range(BPC)]

            # x loads, chunked for pipelining; natural + transposed layouts
            for b in range(BPC):
                for q in range(4):
                    nc.sync.dma_start(
                        xb_sb[b][:, :, q * 1024:(q + 1) * 1024],
                        xb_d[b, :, :, q * 1024:(q + 1) * 1024])
                    nc.sync.dma_start(
                        xt_sb[b][:, q * 8:(q + 1) * 8, :],
                        xt_d[b, :, q * 8:(q + 1) * 8, :])

            def psf(tag, name):
                return psum2.tile([P, 512], f32, tag=tag, name=name)

            def em_iter(b, it):
                mut, mub = mut_cur[b], mub_cur[b]
                g_ps = psf(f"G{b}", f"g{b}_{it}")
                for bank in range(NBK):
                    a_ps = psf("A%d" % (bank % 2), f"a{b}{it}{bank}")
                    for t8 in range(TPB):
                        t = bank * TPB + t8
                        sl = a_ps[:, t8 * K:(t8 + 1) * K]
                        for ct in range(OC):
                            nc.tensor.matmul(
                                sl, xb_sb[b][:, ct, t * P:(t + 1) * P],
                                mut[:, ct, :],
                                start=(ct == 0), stop=False)
                        nc.tensor.matmul(sl, oner_sb[:], mub[:],
                                         start=False, stop=True)
                    et = etp.tile([P, TPB * K], f32, tag="et",
                                  name=f"et{b}{it}{bank}")
                    nc.scalar.activation(et[:], a_ps[:], AF.Exp)
                    if debug_dumps and b == 0 and it == 0 and bank == 0:
                        nc.sync.dma_start(dbg["dbg_et"], et[:])
                    et3 = et[:].rearrange("p (t k) -> p t k", k=K)
                    s8 = smalls.tile([P, TPB], f32, tag="s8", bufs=3)
                    nc.vector.tensor_reduce(s8[:], et3, axis=AX.X, op=ALU.add)
                    nc.vector.reciprocal(s8[:], s8[:])
                    zsl = z_sb[b][:, bank * TPB:(bank + 1) * TPB, :]
                    nc.vector.tensor_tensor(
                        zsl, et3, s8[:, :, None].to_broadcast((P, TPB, K)),
                        ALU.mult)
                # accumulation chains must not interleave: complete each
                # PSUM group before opening the next (PE corrupts otherwise)
                for cj in range(OC):
                    for t in range(NT):
                        nc.tensor.matmul(
                            g_ps[:, cj * K:(cj + 1) * K],
                            xt_sb[b][:, t, cj * P:(cj + 1) * P],
                            z_sb[b][:, t, :],
                            start=(t == 0), stop=(t == NT - 1))
                for t in range(NT):
                    nc.tensor.matmul(g_ps[:1, OC * K:(OC + 1) * K],
                                     onec_sb[:], z_sb[b][:, t, :],
                                     start=(t == 0), stop=(t == NT - 1))
                # ---- mu update tail ----
                gx = smalls.tile([P, OC, K], bf16, tag="gx")
                nc.scalar.copy(gx[:], g_ps[:, :OC * K])
                skr = smalls.tile([1, K], bf16, tag="sk")
                nc.vector.tensor_copy(skr[:], g_ps[:1, OC * K:(OC + 1) * K])
                mur_ps = psf("MU", f"mur{b}{it}")
                for o2 in range(OC):
                    msl = mur_ps[:, o2 * K:(o2 + 1) * K]
                    for ct in range(OC):
                        nc.tensor.matmul(msl,
                                         wst_sb[:, ct, o2 * P:(o2 + 1) * P],
                                         gx[:, ct, :],
                                         start=(ct == 0), stop=False)
                    nc.tensor.matmul(msl, bsr_sb[:, o2 * P:(o2 + 1) * P],
                                     skr[:], start=False, stop=True)
                sq = smalls.tile([P, OC, K], bf16, tag="sq")  # mur^2 <= ~5e3, fp16-safe
                nc.scalar.square(sq[:], mur_ps[:, :OC * K])
                nsl = mur_ps[:1, OC * K:(OC + 1) * K]
                for j in range(OC):
                    nc.tensor.matmul(nsl, onec_sb[:], sq[:, j, :],
                                     start=(j == 0), stop=(j == OC - 1))
                if it < NUM_ITER - 1:
                    nr = smalls.tile([1, K], f32, tag="nr")
                    nc.scalar.activation(nr[:], nsl, AF.Ln)
                    rsr = smalls.tile([1, K], bf16, tag="rsr")
                    nc.scalar.activation(rsr[:], nr[:], AF.Exp, scale=-0.5)
                    rep = mur_ps[:, (OC + 1) * K:(OC + 2) * K]
                    nc.tensor.matmul(rep, oner_sb[:], rsr[:],
                                     start=True, stop=True)
                    rep_sb = smalls.tile([P, K], f32, tag="rep")
                    nc.scalar.copy(rep_sb[:], rep)
                    mun = mutp.tile([P, OC, K], bf16, tag=f"mun{b}",
                                    name=f"mun{b}_{it}")
                    nc.vector.tensor_tensor(
                        mun[:],
                        mur_ps[:, :OC * K].rearrange("p (t k) -> p t k", k=K),
                        rep_sb[:, None, :].to_broadcast((P, OC, K)), ALU.mult)
                    mtn_ps = psf("MU2", f"mtn{b}{it}")
                    for cj in range(OC):
                        msl = mtn_ps[:, cj * K:(cj + 1) * K]
                        for ct in range(OC):
                            nc.tensor.matmul(
                                msl, ws_sb[:, ct, cj * P:(cj + 1) * P],
                                mun[:, ct, :],
                                start=(ct == 0), stop=(ct == OC - 1))
                    mutn = mutp.tile([P, OC, K], bf16, tag=f"mut{b}",
                                     name=f"mut{b}_{it}")
                    nc.scalar.copy(mutn[:], mtn_ps[:, :OC * K])
                    mut_cur[b] = mutn
                    if debug_dumps and b == 0 and it == 0:
                        nc.sync.dma_start(dbg["dbg_gx"], gx[:])
                        nc.sync.dma_start(dbg["dbg_sk"], skr[:])
                        nc.sync.dma_start(dbg["dbg_rsr"], rsr[:])
                        nc.sync.dma_start(dbg["dbg_mun"], mun[:])
                        nsq_sb = smalls.tile([1, K], f32, tag="dbgnsq")
                        nc.vector.tensor_copy(nsq_sb[:], nsl)
                        nc.sync.dma_start(dbg["dbg_nsq"], nsq_sb[:])
                    bsl = mtn_ps[:1, OC * K:(OC + 1) * K]
                    for ct in range(OC):
                        nc.tensor.matmul(bsl, bsc_sb[:, ct:ct + 1],
                                         mun[:, ct, :],
                                         start=(ct == 0), stop=(ct == OC - 1))
                    mubn = smalls.tile([1, K], bf16, tag=f"mub{b}")
                    nc.vector.tensor_copy(mubn[:], bsl)
                    mub_cur[b] = mubn
                else:
                    # rsqrt of col norms as a [K,1] column for the zT scale
                    nr2 = smalls.tile([1, K], f32, tag="nr2")
                    nc.scalar.activation(nr2[:], nsl, AF.Ln)
                    rs_row = smalls.tile([1, K], bf16, tag="rsrow")
                    nc.scalar.activation(rs_row[:], nr2[:], AF.Exp,
                                         scale=-0.5)
                    tc_ps = psum2.tile([P, 512], bf16, tag="TC",
                                       name=f"tc{b}")
                    nc.tensor.transpose(tc_ps[:K, 0:1], rs_row[:],
                                        idb[:1, :1])
                    rcol = smalls.tile([K, 1], f32, tag=f"rs{b}", bufs=1)
                    nc.vector.tensor_copy(rcol[:], tc_ps[:K, 0:1])
                    rs_col[b] = rcol
                    murs = smalls.tile([P, OC, K], bf16, tag="murs")
                    nc.vector.tensor_copy(murs[:], mur_ps[:, :OC * K])
                    for j in range(OC):
                        nc.tensor.transpose(tc_ps[:K, j * P:(j + 1) * P],
                                            murs[:, j, :], idb[:])
                    mt = mutp.tile([K, C], bf16, tag=f"muT{b}", bufs=1,
                                   name=f"muT{b}")
                    nc.scalar.copy(mt[:], tc_ps[:K, :])
                    muT_sb[b] = mt

            if debug_dumps:
                nc.sync.dma_start(dbg["dbg_xb"], xb_sb[0][:, :, :1024])
                nc.sync.dma_start(dbg["dbg_xt"], xt_sb[0][:])
                nc.sync.dma_start(dbg["dbg_mut0"], mut0_sb[:])
            for (b, it) in [(0, 0), (0, 1), (1, 0), (0, 2), (1, 1),
                            (1, 2)]:
                em_iter(b, it)
                if debug_dumps and b == 0 and it == 0:
                    nc.sync.dma_start(dbg["dbg_z"], z_sb[0][:])
                    nc.sync.dma_start(dbg["dbg_mut"], mut_cur[0][:])
            if debug_dumps:
                nc.sync.dma_start(dbg["dbg_muT"], muT_sb[0][:])
                nc.sync.dma_start(dbg["dbg_rs"], rs_col[0][:])

        # ================= L3: y2 / head / BN / final ====================
        with ExitStack() as l3:
            hbig = l3.enter_context(tc.tile_pool(name="hbig", bufs=1))
            ztp = l3.enter_context(tc.tile_pool(name="ztp", bufs=2))
            ry2p = l3.enter_context(tc.tile_pool(name="ry2p", bufs=2))
            junkp = l3.enter_context(tc.tile_pool(name="junkp", bufs=2))
            fstage = l3.enter_context(tc.tile_pool(name="fstage", bufs=5))
            psum3 = l3.enter_context(tc.tile_pool(name="psum3", bufs=1,
                                                  space="PSUM"))

            hwt_sb = consts.tile([P, OC, C], bf16, name="hwt_sb")
            nc.sync.dma_start(hwt_sb[:], hwt_d)

            h_of = [hbig.tile([P, OC, N], bf16, name=f"h{b}")
                    for b in range(BPC)]

            NCH = N // 512

            def l3_front(b, ch):
                zt_ps = psum3.tile([P, 512], bf16, tag="ZT", bufs=2,
                                   name=f"zt{b}{ch}")
                for j in range(4):
                    nc.tensor.transpose(zt_ps[:K, j * P:(j + 1) * P],
                                        z_sb[b][:, ch * 4 + j, :], idb[:])
                zts = ztp.tile([K, 512], bf16, tag="zt")
                nc.vector.tensor_scalar(zts[:], zt_ps[:K, :],
                                        rs_col[b], None, ALU.mult)
                return zts

            def emit_sq(b, pair):
                qcol = b * (NCH // 2) + pair
                for o2 in range(OC):
                    junk = junkp.tile([P, 1024], bf16, tag="junk")
                    nc.scalar.activation(
                        junk[:],
                        h_of[b][:, o2, pair * 1024:(pair + 1) * 1024],
                        AF.Square, accum_out=sq_acc[:, o2, qcol:qcol + 1])

            chunks = [(b, ch) for b in range(BPC) for ch in range(NCH)]
            zts_prev = l3_front(*chunks[0])
            for gi, (b, ch) in enumerate(chunks):
                zts = zts_prev
                if gi + 1 < len(chunks):
                    zts_prev = l3_front(*chunks[gi + 1])
                ry2 = ry2p.tile([P, OC, 512], bf16, tag="ry2")
                for ot in range(OC):
                    y2_ps = psum3.tile([P, 512], f32, tag="Y2%d" % (ot % 2),
                                       name=f"y2{b}{ch}{ot}")
                    nc.tensor.matmul(y2_ps[:],
                                     muT_sb[b][:, ot * P:(ot + 1) * P],
                                     zts[:], start=True, stop=True)
                    if (ch * OC + ot) % 4 == 0:
                        nc.vector.tensor_scalar(ry2[:, ot, :], y2_ps[:],
                                                0.0, None, ALU.max)
                    else:
                        nc.scalar.activation(ry2[:, ot, :], y2_ps[:],
                                             AF.Relu)
                acol = b * NCH + ch
                for o2 in range(OC):
                    h_ps = psum3.tile([P, 512], f32, tag="H%d" % (o2 % 2),
                                      name=f"h{b}{ch}{o2}")
                    for oc in range(OC):
                        nc.tensor.matmul(
                            h_ps[:], hwt_sb[:, oc, o2 * P:(o2 + 1) * P],
                            ry2[:, oc, :],
                            start=(oc == 0), stop=(oc == OC - 1))
                    dap = h_of[b][:, o2, ch * 512:(ch + 1) * 512]
                    nc.vector.tensor_scalar(
                        dap, h_ps[:], 0.0, 0.0, ALU.add, ALU.add,
                        accum_out=sum_acc[:, o2, acol:acol + 1])
                # squares only feed BN stats: run them a chunk-pair behind
                # so they never sit ahead of the relus in the Act queue
                if gi >= 2 and gi % 2 == 1:
                    emit_sq(*divmod((gi - 2) // 2, NCH // 2))
                if gi == BPC * NCH - 1:
                    emit_sq(*divmod(gi // 2, NCH // 2))

            if debug_dumps:
                nc.sync.dma_start(dbg["dbg_h"], h_of[0][:])
            # ---- BN stats: aggregate, AllReduce, affine coefficients ----
            pack = statp.tile([P, 2 * OC], f32)
            packv = pack[:].rearrange("p (o two) -> p o two", two=2)
            nc.vector.tensor_reduce(packv[:, :, 0:1], sum_acc[:], axis=AX.X,
                                    op=ALU.add)
            nc.vector.tensor_reduce(packv[:, :, 1:2], sq_acc[:], axis=AX.X,
                                    op=ALU.add)
            nc.sync.dma_start(st_in_d[:], pack[:])
            if use_collective:
                nc.gpsimd.collective_compute(
                    "AllReduce", ALU.add,
                    replica_groups=[list(range(n_devices))],
                    ins=[st_in_d[:]],
                    outs=[st_out_d[:]],
                )
            else:
                nc.sync.dma_start(st_out_d[:], st_in_d[:])
            red = statp.tile([P, 2 * OC], f32)
            nc.sync.dma_start(red[:], st_out_d[:])
            a_sb = statp.tile([P, OC], f32)
            b2_sb = statp.tile([P, OC], f32)
            inv_nb = 1.0 / float(B * N)
            redv = red[:].rearrange("p (o two) -> p o two", two=2)
            mean = statp.tile([P, OC], f32)
            nc.vector.tensor_scalar(mean[:], redv[:, :, 0], inv_nb, None,
                                    ALU.mult)
            var = statp.tile([P, OC], f32)
            nc.vector.tensor_scalar(var[:], redv[:, :, 1], inv_nb, None,
                                    ALU.mult)
            msq = statp.tile([P, OC], f32)
            nc.vector.tensor_tensor(msq[:], mean[:], mean[:], ALU.mult)
            nc.vector.tensor_tensor(var[:], var[:], msq[:], ALU.subtract)
            # rstd = exp(-0.5*ln(var+eps))
            nc.vector.tensor_scalar(var[:], var[:], BN_EPS, None, ALU.add)
            nc.scalar.activation(var[:], var[:], AF.Ln)
            nc.scalar.activation(var[:], var[:], AF.Exp, scale=-0.5)
            nc.vector.tensor_tensor(a_sb[:], gm_sb[:], var[:], ALU.mult)
            nc.vector.tensor_tensor(msq[:], mean[:], a_sb[:], ALU.mult)
            nc.vector.tensor_tensor(b2_sb[:], bt_sb[:], msq[:],
                                    ALU.subtract)

            if debug_dumps:
                nc.sync.dma_start(dbg["dbg_pack"], pack[:])
                abp = statp.tile([P, 2 * OC], f32, name="abp")
                nc.vector.tensor_copy(abp[:, :OC], a_sb[:])
                nc.vector.tensor_copy(abp[:, OC:], b2_sb[:])
                nc.sync.dma_start(dbg["dbg_ab"], abp[:])
            # ---- final: out = relu(h*a + b2 + x) ----
            fchunks = [(b, o2, fc) for b in range(BPC)
                       for o2 in range(OC) for fc in range(FCH)]

            def f_front(fi, b, o2, fc):
                hap = h_of[b][:, o2, fc * 1024:(fc + 1) * 1024]
                xap = xb_sb[b][:, o2, fc * 1024:(fc + 1) * 1024]
                t1 = fstage.tile([P, 1024], bf16, tag="t1")
                nc.vector.tensor_scalar(t1[:], hap, a_sb[:, o2:o2 + 1],
                                        b2_sb[:, o2:o2 + 1],
                                        ALU.mult, ALU.add)
                t2 = fstage.tile([P, 1024], bf16, tag="t2")
                if fi % 4 == 3:
                    nc.gpsimd.tensor_tensor(t2[:], t1[:], xap, ALU.add)
                else:
                    nc.vector.tensor_tensor(t2[:], t1[:], xap, ALU.add)
                return t2

            prev = f_front(0, *fchunks[0])
            for fi, (b, o2, fc) in enumerate(fchunks):
                t2 = prev
                if fi + 1 < len(fchunks):
                    prev = f_front(fi + 1, *fchunks[fi + 1])
                otile = fstage.tile([P, 1024], bf16, tag="ot")
                if fi % 4 == 0:
                    nc.vector.tensor_scalar(otile[:], t2[:],
                                            0.0, None, ALU.max)
                else:
                    nc.scalar.activation(otile[:], t2[:], AF.Relu)
                nc.sync.dma_start(
                    out_d[b, o2 * P:(o2 + 1) * P,
                          fc * 1024:(fc + 1) * 1024], otile[:])

    _hoist_extra_waits(nc)
    return nc


_ENGINE_SEM_PREFIX = {
    "EngineType.PE": "PE_",
    "EngineType.Activation": "Activation_",
    "EngineType.DVE": "DVE_",
    "EngineType.Pool": "Pool_",
    "EngineType.SP": "SP_",
}


def _hoist_extra_waits(nc):
    """This walrus build rejects compute-engine instructions carrying more
    than one sync wait. Engine queues are strict FIFO, so (a) an
    instruction waiting on its own engine's semaphore is always already
    satisfied -> drop it; (b) any extra waits can be hoisted onto NoOp
    instructions injected just before, one wait each -- identical
    semantics."""
    import concourse.mybir as mybir
    nid = 0
    for blk in nc.m.functions[0].blocks:
        out = []
        changed = False
        for i in blk.instructions:
            si = getattr(i, "sync_info", None)
            eng = str(getattr(i, "engine", None))
            waits = list(si.on_wait) if si and si.on_wait else []
            if len(waits) > 1 and eng in _ENGINE_SEM_PREFIX:
                selfp = _ENGINE_SEM_PREFIX[eng]
                waits = [w for w in waits if not w.ant_name.startswith(selfp)]
                for w in waits[:-1]:
                    nid += 1
                    out.append(mybir.InstNoOp(
                        name=f"I-waitnop-{nid}",
                        engine=i.engine,
                        sync_info=mybir.SyncInfo(on_wait=[w], on_update=[]),
                        bass_nofuse=True,
                    ))
                i.sync_info = mybir.SyncInfo(
                    on_wait=waits[-1:], on_update=list(si.on_update or []))
                changed = True
            out.append(i)
        if changed:
            blk.instructions = out


def get_nc():
    if "nc" not in _cache:
        _cache["nc"] = _build_nc()
    return _cache["nc"]


def run(inputs_by_core, trace=False):
    from concourse.bass_utils import run_bass_kernel_spmd
    nc = get_nc()
    return run_bass_kernel_spmd(nc, inputs_by_core, list(range(NCORES)),
                                trace=trace)


def make_in_maps(x, mu, stem_w, stem_b, head_w, head_b, bn_gamma, bn_beta):
    bf16 = np.float16

    x = np.ascontiguousarray(np.asarray(x, np.float32)).reshape(B, C, N)
    mu = np.asarray(mu, np.float32)
    stem_w = np.asarray(stem_w, np.float32)
    stem_b = np.asarray(stem_b, np.float32)
    head_w = np.asarray(head_w, np.float32)

    def tile_rows(m):  # (C, F) -> (P, OC, F) with row t*P+p -> [p, t]
        return np.ascontiguousarray(
            m.reshape(OC, P, -1).transpose(1, 0, 2).astype(bf16))

    mut0 = stem_w.T @ mu                    # (C, K)
    mub0 = (mu.T @ stem_b)[None, :]         # (1, K)

    common = {
        "ws": tile_rows(stem_w),
        "wst": tile_rows(stem_w.T),
        "hwt": tile_rows(head_w.T),
        "mut0": tile_rows(mut0),
        "mub0": np.ascontiguousarray(mub0.astype(bf16)),
        "bsr": np.ascontiguousarray(stem_b[None, :].astype(bf16)),
        "bsc": np.ascontiguousarray(
            stem_b.reshape(OC, P).T.astype(bf16)),
        "onec": np.ones((P, 1), bf16),
        "oner": np.ones((1, P), bf16),
        "gm": np.ascontiguousarray(
            np.asarray(bn_gamma, np.float32).reshape(OC, P).T),
        "bt": np.ascontiguousarray(
            np.asarray(bn_beta, np.float32).reshape(OC, P).T),
    }
    maps = []
    for i in range(NCORES):
        xc = x[i * BPC:(i + 1) * BPC]                     # (BPC, C, N)
        xbt = xc.reshape(BPC, OC, P, N).transpose(0, 2, 1, 3)
        xtt = (xc.transpose(0, 2, 1)                      # (BPC, N, C)
               .reshape(BPC, NT, P, C).transpose(0, 2, 1, 3))
        maps.append({
            "xb": np.ascontiguousarray(xbt.astype(bf16)),
            "xt": np.ascontiguousarray(xtt.astype(bf16)),
            **common,
        })
    return maps


def kernel(x, mu, stem_w, stem_b, head_w, head_b, bn_gamma, bn_beta):
    in_maps = make_in_maps(x, mu, stem_w, stem_b, head_w, head_b,
                           bn_gamma, bn_beta)
    res = run(in_maps, trace=False)
    out = np.concatenate([res.results[i]["out"] for i in range(NCORES)],
                         axis=0)
    return out.reshape(B, C, 64, 64).astype(np.float32)


# revision 55
# speedup vs baseline: 1.0183x; 1.0183x over previous
"""EMAttention2d (vq_codebook) Trainium2 kernel, v2.

Data parallel over batch: 16 images -> 8 cores x 2 images. BN batch stats
cross-core reduced with a tiny AllReduce.

Key layout change vs v1: the EM loop works in pixel-partition layout so
softmax needs no transposes, and all big matmuls keep a full 128-row
output partition with bf16 moving operands (1 cycle/row on PE):

  per image (X = x[b] as (C,N), host also supplies X^T; both bf16):
    mu~ = Ws^T mu          (C,K)  [stem folded into codebook; host for it0]
    mub = mu^T bs          (K,)   [host for it0]
    repeat 3x:
      A[n,k]  = X^T mu~ + 1 (x) mub    - 4 chunk matmuls + rank-1 per tile
      E       = exp(A)                 (N,K) bf16, pixel-partition
      z       = E / rowsum(E)          softmax over free dim k
      Gx[c,k] = sum_n X[c,n] z[n,k]    - lhsT = X^T tiles, rhs = z tiles
      sk      = 1^T z
      muR     = Ws Gx + bs (x) sk      (C,K) natural layout
      mu      = muR / ||col||_2        - norm via ones^T muR^2 matmuls
    y2   = mu z^T   (relu) -> head Hw  - z^T via 1c/r bf16 PE transposes
  BN over batch (AllReduce of per-channel sum/sumsq), then
    out = relu(h*a + b2 + x),  a = gamma*rstd, b2 = beta - mean*a

x stays resident in SBUF (bf16) so the final pass reloads nothing.
"""

import sys

for _p in ("/opt/trn_rl_repo",):
    if _p not in sys.path:
        sys.path.insert(0, _p)

import numpy as np

B, C, N, K = 16, 512, 4096, 64
NCORES = 8
BPC = B // NCORES  # images per core
P = 128
OC = C // P   # 4 chunks of channels
NT = N // P   # 32 pixel tiles
NBK = 4       # A-banks per EM iteration
TPB = NT // NBK  # pixel tiles per bank (8)
FCH = N // 1024  # final-pass chunks per (img, o2)
BN_EPS = 1e-5
NUM_ITER = 3

_cache = {}


def _build_nc(n_devices=NCORES, use_collective=True, debug_dumps=False):
    import concourse.bass as bass
    import concourse.mybir as mybir
    import concourse.tile as tile
    from concourse.masks import make_identity
    from contextlib import ExitStack

    dt = mybir.dt
    f32 = dt.float32
    bf16 = dt.float16  # fp16 storage: 8x finer rounding than bf16, same engine rates
    bfr = dt.bfloat16
    AF = mybir.ActivationFunctionType
    ALU = mybir.AluOpType
    AX = mybir.AxisListType

    nc = bass.Bass("TRN2", target_bir_lowering=False, debug=False,
                   num_devices=n_devices)

    xb_d = nc.dram_tensor("xb", [BPC, P, OC, N], bf16, kind="ExternalInput").ap()
    xt_d = nc.dram_tensor("xt", [BPC, P, NT, C], bf16, kind="ExternalInput").ap()
    ws_d = nc.dram_tensor("ws", [P, OC, C], bf16, kind="ExternalInput").ap()
    wst_d = nc.dram_tensor("wst", [P, OC, C], bf16, kind="ExternalInput").ap()
    hwt_d = nc.dram_tensor("hwt", [P, OC, C], bf16, kind="ExternalInput").ap()
    mut0_d = nc.dram_tensor("mut0", [P, OC, K], bf16, kind="ExternalInput").ap()
    mub0_d = nc.dram_tensor("mub0", [1, K], bf16, kind="ExternalInput").ap()
    bsr_d = nc.dram_tensor("bsr", [1, C], bf16, kind="ExternalInput").ap()
    bsc_d = nc.dram_tensor("bsc", [P, OC], bf16, kind="ExternalInput").ap()
    onec_d = nc.dram_tensor("onec", [P, 1], bf16, kind="ExternalInput").ap()
    oner_d = nc.dram_tensor("oner", [1, P], bf16, kind="ExternalInput").ap()
    gm_d = nc.dram_tensor("gm", [P, OC], f32, kind="ExternalInput").ap()
    bt_d = nc.dram_tensor("bt", [P, OC], f32, kind="ExternalInput").ap()
    out_d = nc.dram_tensor("out", [BPC, C, N], bf16, kind="ExternalOutput").ap()
    st_in_d = nc.dram_tensor("stats_in", [P, 2 * OC], f32).ap()
    st_out_d = nc.dram_tensor("stats_out", [P, 2 * OC], f32,
                              addr_space="Shared").ap()
    if debug_dumps:
        bf16_ = dt.float16
        dbg = {
            "dbg_id": nc.dram_tensor("dbg_id", [P, P], bf16_,
                                     kind="ExternalOutput").ap(),
            "dbg_z": nc.dram_tensor("dbg_z", [P, NT, K], bf16_,
                                    kind="ExternalOutput").ap(),
            "dbg_mut": nc.dram_tensor("dbg_mut", [P, OC, K], bf16_,
                                      kind="ExternalOutput").ap(),
            "dbg_muT": nc.dram_tensor("dbg_muT", [K, C], bf16_,
                                      kind="ExternalOutput").ap(),
            "dbg_rs": nc.dram_tensor("dbg_rs", [K, 1], f32,
                                     kind="ExternalOutput").ap(),
            "dbg_h": nc.dram_tensor("dbg_h", [P, OC, N], bf16_,
                                    kind="ExternalOutput").ap(),
            "dbg_pack": nc.dram_tensor("dbg_pack", [P, 2 * OC], f32,
                                       kind="ExternalOutput").ap(),
            "dbg_ab": nc.dram_tensor("dbg_ab", [P, 2 * OC], f32,
                                     kind="ExternalOutput").ap(),
            "dbg_xb": nc.dram_tensor("dbg_xb", [P, OC, 1024], bf16_,
                                     kind="ExternalOutput").ap(),
            "dbg_xt": nc.dram_tensor("dbg_xt", [P, NT, C], bf16_,
                                     kind="ExternalOutput").ap(),
            "dbg_et": nc.dram_tensor("dbg_et", [P, 512], f32,
                                     kind="ExternalOutput").ap(),
            "dbg_mut0": nc.dram_tensor("dbg_mut0", [P, OC, K], bf16_,
                                       kind="ExternalOutput").ap(),
            "dbg_gx": nc.dram_tensor("dbg_gx", [P, OC, K], bf16_,
                                     kind="ExternalOutput").ap(),
            "dbg_sk": nc.dram_tensor("dbg_sk", [1, K], bf16_,
                                     kind="ExternalOutput").ap(),
            "dbg_rsr": nc.dram_tensor("dbg_rsr", [1, K], bf16_,
                                      kind="ExternalOutput").ap(),
            "dbg_mun": nc.dram_tensor("dbg_mun", [P, OC, K], bf16_,
                                      kind="ExternalOutput").ap(),
            "dbg_nsq": nc.dram_tensor("dbg_nsq", [1, K], f32,
                                      kind="ExternalOutput").ap(),
        }

    with tile.TileContext(nc) as tc, ExitStack() as ctx:
        consts = ctx.enter_context(tc.tile_pool(name="consts", bufs=1))
        xbig = ctx.enter_context(tc.tile_pool(name="xbig", bufs=1))
        zpool = ctx.enter_context(tc.tile_pool(name="zpool", bufs=1))
        mutp = ctx.enter_context(tc.tile_pool(name="mutp", bufs=2))
        smalls = ctx.enter_context(tc.tile_pool(name="smalls", bufs=2))
        statp = ctx.enter_context(tc.tile_pool(name="statp", bufs=1))

        # ---- constants ----
        idb = consts.tile([P, P], bf16)
        make_identity(nc, idb[:])
        if debug_dumps:
            nc.sync.dma_start(dbg["dbg_id"], idb[:])
        ws_sb = consts.tile([P, OC, C], bf16)
        nc.sync.dma_start(ws_sb[:], ws_d)
        wst_sb = consts.tile([P, OC, C], bf16)
        nc.sync.dma_start(wst_sb[:], wst_d)
        mut0_sb = consts.tile([P, OC, K], bf16)
        nc.sync.dma_start(mut0_sb[:], mut0_d)
        mub0_sb = consts.tile([1, K], bf16)
        nc.sync.dma_start(mub0_sb[:], mub0_d)
        bsr_sb = consts.tile([1, C], bf16)
        nc.sync.dma_start(bsr_sb[:], bsr_d)
        bsc_sb = consts.tile([P, OC], bf16)
        nc.sync.dma_start(bsc_sb[:], bsc_d)
        onec_sb = consts.tile([P, 1], bf16)
        nc.sync.dma_start(onec_sb[:], onec_d)
        oner_sb = consts.tile([1, P], bf16)
        nc.sync.dma_start(oner_sb[:], oner_d)
        gm_sb = consts.tile([P, OC], f32)
        nc.sync.dma_start(gm_sb[:], gm_d)
        bt_sb = consts.tile([P, OC], f32)
        nc.sync.dma_start(bt_sb[:], bt_d)
        eps_sb = consts.tile([P, 1], f32)
        nc.vector.memset(eps_sb[:], BN_EPS)

        xb_sb = [xbig.tile([P, OC, N], bf16, name=f"xb{b}") for b in range(BPC)]
        z_sb = [zpool.tile([P, NT, K], bf16, name=f"z{b}") for b in range(BPC)]

        sum_acc = statp.tile([P, OC, BPC * (N // 512)], f32)
        sq_acc = statp.tile([P, OC, BPC * (N // 1024)], f32)

        mut_cur = [mut0_sb, mut0_sb]
        mub_cur = [mub0_sb, mub0_sb]
        rs_col = [None] * BPC
        muT_sb = [None] * BPC

        # ================= EM phase (both images, interleaved) ============
        with ExitStack() as l2:
            xtp = l2.enter_context(tc.tile_pool(name="xtp", bufs=1))
            etp = l2.enter_context(tc.tile_pool(name="etp", bufs=4))
            psum2 = l2.enter_context(tc.tile_pool(name="psum2", bufs=1,
                                                  space="PSUM"))

            xt_sb = [xtp.tile([P, NT, C], bf16, name=f"xt{b}")
                     for b in range(BPC)]

            # x loads, chunked for pipelining; natural + transposed layouts
            for b in range(BPC):
                for q in range(4):
                    nc.sync.dma_start(
                        xb_sb[b][:, :, q * 1024:(q + 1) * 1024],
                        xb_d[b, :, :, q * 1024:(q + 1) * 1024])
                    nc.sync.dma_start(
                        xt_sb[b][:, q * 8:(q + 1) * 8, :],
                        xt_d[b, :, q * 8:(q + 1) * 8, :])

            def psf(tag, name):
                return psum2.tile([P, 512], f32, tag=tag, name=name)

            def em_iter(b, it):
                mut, mub = mut_cur[b], mub_cur[b]
                g_ps = psf(f"G{b}", f"g{b}_{it}")
                for bank in range(NBK):
                    a_ps = psf("A%d" % (bank % 2), f"a{b}{it}{bank}")
                    for t8 in range(TPB):
                        t = bank * TPB + t8
                        sl = a_ps[:, t8 * K:(t8 + 1) * K]
                        for ct in range(OC):
                            nc.tensor.matmul(
                                sl, xb_sb[b][:, ct, t * P:(t + 1) * P],
                                mut[:, ct, :],
                                start=(ct == 0), stop=False)
                        nc.tensor.matmul(sl, oner_sb[:], mub[:],
                                         start=False, stop=True)
                    et = etp.tile([P, TPB * K], f32, tag="et",
                                  name=f"et{b}{it}{bank}")
                    nc.scalar.activation(et[:], a_ps[:], AF.Exp)
                    if debug_dumps and b == 0 and it == 0 and bank == 0:
                        nc.sync.dma_start(dbg["dbg_et"], et[:])
                    et3 = et[:].rearrange("p (t k) -> p t k", k=K)
                    s8 = smalls.tile([P, TPB], f32, tag="s8", bufs=3)
                    nc.vector.tensor_reduce(s8[:], et3, axis=AX.X, op=ALU.add)
                    nc.vector.reciprocal(s8[:], s8[:])
                    zsl = z_sb[b][:, bank * TPB:(bank + 1) * TPB, :]
                    nc.vector.tensor_tensor(
                        zsl, et3, s8[:, :, None].to_broadcast((P, TPB, K)),
                        ALU.mult)
                # accumulation chains must not interleave: complete each
                # PSUM group before opening the next (PE corrupts otherwise)
                for cj in range(OC):
                    for t in range(NT):
                        nc.tensor.matmul(
                            g_ps[:, cj * K:(cj + 1) * K],
                            xt_sb[b][:, t, cj * P:(cj + 1) * P],
                            z_sb[b][:, t, :],
                            start=(t == 0), stop=(t == NT - 1))
                for t in range(NT):
                    nc.tensor.matmul(g_ps[:1, OC * K:(OC + 1) * K],
                                     onec_sb[:], z_sb[b][:, t, :],
                                     start=(t == 0), stop=(t == NT - 1))
                # ---- mu update tail ----
                gx = smalls.tile([P, OC, K], bf16, tag="gx")
                nc.scalar.copy(gx[:], g_ps[:, :OC * K])
                skr = smalls.tile([1, K], bf16, tag="sk")
                nc.vector.tensor_copy(skr[:], g_ps[:1, OC * K:(OC + 1) * K])
                mur_ps = psf("MU", f"mur{b}{it}")
                for o2 in range(OC):
                    msl = mur_ps[:, o2 * K:(o2 + 1) * K]
                    for ct in range(OC):
                        nc.tensor.matmul(msl,
                                         wst_sb[:, ct, o2 * P:(o2 + 1) * P],
                                         gx[:, ct, :],
                                         start=(ct == 0), stop=False)
                    nc.tensor.matmul(msl, bsr_sb[:, o2 * P:(o2 + 1) * P],
                                     skr[:], start=False, stop=True)
                sq = smalls.tile([P, OC, K], bf16, tag="sq")  # mur^2 <= ~5e3, fp16-safe
                nc.scalar.square(sq[:], mur_ps[:, :OC * K])
                nsl = mur_ps[:1, OC * K:(OC + 1) * K]
                for j in range(OC):
                    nc.tensor.matmul(nsl, onec_sb[:], sq[:, j, :],
                                     start=(j == 0), stop=(j == OC - 1))
                if it < NUM_ITER - 1:
                    nr = smalls.tile([1, K], f32, tag="nr")
                    nc.scalar.activation(nr[:], nsl, AF.Ln)
                    rsr = smalls.tile([1, K], bf16, tag="rsr")
                    nc.scalar.activation(rsr[:], nr[:], AF.Exp, scale=-0.5)
                    rep = mur_ps[:, (OC + 1) * K:(OC + 2) * K]
                    nc.tensor.matmul(rep, oner_sb[:], rsr[:],
                                     start=True, stop=True)
                    rep_sb = smalls.tile([P, K], f32, tag="rep")
                    nc.scalar.copy(rep_sb[:], rep)
                    mun = mutp.tile([P, OC, K], bf16, tag=f"mun{b}",
                                    name=f"mun{b}_{it}")
                    nc.vector.tensor_tensor(
                        mun[:],
                        mur_ps[:, :OC * K].rearrange("p (t k) -> p t k", k=K),
                        rep_sb[:, None, :].to_broadcast((P, OC, K)), ALU.mult)
                    mtn_ps = psf("MU2", f"mtn{b}{it}")
                    for cj in range(OC):
                        msl = mtn_ps[:, cj * K:(cj + 1) * K]
                        for ct in range(OC):
                            nc.tensor.matmul(
                                msl, ws_sb[:, ct, cj * P:(cj + 1) * P],
                                mun[:, ct, :],
                                start=(ct == 0), stop=(ct == OC - 1))
                    mutn = mutp.tile([P, OC, K], bf16, tag=f"mut{b}",
                                     name=f"mut{b}_{it}")
                    nc.scalar.copy(mutn[:], mtn_ps[:, :OC * K])
                    mut_cur[b] = mutn
                    if debug_dumps and b == 0 and it == 0:
                        nc.sync.dma_start(dbg["dbg_gx"], gx[:])
                        nc.sync.dma_start(dbg["dbg_sk"], skr[:])
                        nc.sync.dma_start(dbg["dbg_rsr"], rsr[:])
                        nc.sync.dma_start(dbg["dbg_mun"], mun[:])
                        nsq_sb = smalls.tile([1, K], f32, tag="dbgnsq")
                        nc.vector.tensor_copy(nsq_sb[:], nsl)
                        nc.sync.dma_start(dbg["dbg_nsq"], nsq_sb[:])
                    bsl = mtn_ps[:1, OC * K:(OC + 1) * K]
                    for ct in range(OC):
                        nc.tensor.matmul(bsl, bsc_sb[:, ct:ct + 1],
                                         mun[:, ct, :],
                                         start=(ct == 0), stop=(ct == OC - 1))
                    mubn = smalls.tile([1, K], bf16, tag=f"mub{b}")
                    nc.vector.tensor_copy(mubn[:], bsl)
                    mub_cur[b] = mubn
                else:
                    # rsqrt of col norms as a [K,1] column for the zT scale
                    nr2 = smalls.tile([1, K], f32, tag="nr2")
                    nc.scalar.activation(nr2[:], nsl, AF.Ln)
                    rs_row = smalls.tile([1, K], bf16, tag="rsrow")
                    nc.scalar.activation(rs_row[:], nr2[:], AF.Exp,
                                         scale=-0.5)
                    tc_ps = psum2.tile([P, 512], bf16, tag="TC",
                                       name=f"tc{b}")
                    nc.tensor.transpose(tc_ps[:K, 0:1], rs_row[:],
                                        idb[:1, :1])
                    rcol = smalls.tile([K, 1], f32, tag=f"rs{b}", bufs=1)
                    nc.vector.tensor_copy(rcol[:], tc_ps[:K, 0:1])
                    rs_col[b] = rcol
                    murs = smalls.tile([P, OC, K], bf16, tag="murs")
                    nc.vector.tensor_copy(murs[:], mur_ps[:, :OC * K])
                    for j in range(OC):
                        nc.tensor.transpose(tc_ps[:K, j * P:(j + 1) * P],
                                            murs[:, j, :], idb[:])
                    mt = mutp.tile([K, C], bf16, tag=f"muT{b}", bufs=1,
                                   name=f"muT{b}")
                    nc.scalar.copy(mt[:], tc_ps[:K, :])
                    muT_sb[b] = mt

            if debug_dumps:
                nc.sync.dma_start(dbg["dbg_xb"], xb_sb[0][:, :, :1024])
                nc.sync.dma_start(dbg["dbg_xt"], xt_sb[0][:])
                nc.sync.dma_start(dbg["dbg_mut0"], mut0_sb[:])
            for (b, it) in [(0, 0), (0, 1), (1, 0), (0, 2), (1, 1),
                            (1, 2)]:
                em_iter(b, it)
                if debug_dumps and b == 0 and it == 0:
                    nc.sync.dma_start(dbg["dbg_z"], z_sb[0][:])
                    nc.sync.dma_start(dbg["dbg_mut"], mut_cur[0][:])
            if debug_dumps:
                nc.sync.dma_start(dbg["dbg_muT"], muT_sb[0][:])
                nc.sync.dma_start(dbg["dbg_rs"], rs_col[0][:])

        # ================= L3: y2 / head / BN / final ====================
        with ExitStack() as l3:
            hbig = l3.enter_context(tc.tile_pool(name="hbig", bufs=1))
            ztp = l3.enter_context(tc.tile_pool(name="ztp", bufs=4))
            ry2p = l3.enter_context(tc.tile_pool(name="ry2p", bufs=2))
            junkp = l3.enter_context(tc.tile_pool(name="junkp", bufs=1))
            fstage = l3.enter_context(tc.tile_pool(name="fstage", bufs=5))
            psum3 = l3.enter_context(tc.tile_pool(name="psum3", bufs=1,
                                                  space="PSUM"))

            hwt_sb = consts.tile([P, OC, C], bf16, name="hwt_sb")
            nc.sync.dma_start(hwt_sb[:], hwt_d)

            h_of = [hbig.tile([P, OC, N], bf16, name=f"h{b}")
                    for b in range(BPC)]

            NCH = N // 512

            def l3_front(b, ch):
                zt_ps = psum3.tile([P, 512], bf16, tag="ZT", bufs=2,
                                   name=f"zt{b}{ch}")
                for j in range(4):
                    nc.tensor.transpose(zt_ps[:K, j * P:(j + 1) * P],
                                        z_sb[b][:, ch * 4 + j, :], idb[:])
                zts = ztp.tile([K, 512], bf16, tag="zt")
                nc.vector.tensor_scalar(zts[:], zt_ps[:K, :],
                                        rs_col[b], None, ALU.mult)
                return zts

            def emit_sq(b, pair):
                qcol = b * (NCH // 2) + pair
                for o2 in range(OC):
                    junk = junkp.tile([P, 1024], bf16, tag="junk")
                    nc.scalar.activation(
                        junk[:],
                        h_of[b][:, o2, pair * 1024:(pair + 1) * 1024],
                        AF.Square, accum_out=sq_acc[:, o2, qcol:qcol + 1])

            chunks = [(b, ch) for b in range(BPC) for ch in range(NCH)]
            zts_prev = l3_front(*chunks[0])
            for gi, (b, ch) in enumerate(chunks):
                zts = zts_prev
                if gi + 1 < len(chunks):
                    zts_prev = l3_front(*chunks[gi + 1])
                ry2 = ry2p.tile([P, OC, 512], bf16, tag="ry2")
                for ot in range(OC):
                    y2_ps = psum3.tile([P, 512], f32, tag="Y2%d" % (ot % 2), bufs=2,
                                       name=f"y2{b}{ch}{ot}")
                    nc.tensor.matmul(y2_ps[:],
                                     muT_sb[b][:, ot * P:(ot + 1) * P],
                                     zts[:], start=True, stop=True)
                    if (ch * OC + ot) % 4 == 0:
                        nc.vector.tensor_scalar(ry2[:, ot, :], y2_ps[:],
                                                0.0, None, ALU.max)
                    else:
                        nc.scalar.activation(ry2[:, ot, :], y2_ps[:],
                                             AF.Relu)
                acol = b * NCH + ch
                for o2 in range(OC):
                    h_ps = psum3.tile([P, 512], f32, tag="H%d" % (o2 % 2),
                                      name=f"h{b}{ch}{o2}")
                    for oc in range(OC):
                        nc.tensor.matmul(
                            h_ps[:], hwt_sb[:, oc, o2 * P:(o2 + 1) * P],
                            ry2[:, oc, :],
                            start=(oc == 0), stop=(oc == OC - 1))
                    dap = h_of[b][:, o2, ch * 512:(ch + 1) * 512]
                    nc.vector.tensor_scalar(
                        dap, h_ps[:], 0.0, 0.0, ALU.add, ALU.add,
                        accum_out=sum_acc[:, o2, acol:acol + 1])
                # squares only feed BN stats: run them a chunk-pair behind
                # so they never sit ahead of the relus in the Act queue
                if gi >= 2 and gi % 2 == 1:
                    emit_sq(*divmod((gi - 2) // 2, NCH // 2))
                if gi == BPC * NCH - 1:
                    emit_sq(*divmod(gi // 2, NCH // 2))

            if debug_dumps:
                nc.sync.dma_start(dbg["dbg_h"], h_of[0][:])
            # ---- BN stats: aggregate, AllReduce, affine coefficients ----
            pack = statp.tile([P, 2 * OC], f32)
            packv = pack[:].rearrange("p (o two) -> p o two", two=2)
            nc.vector.tensor_reduce(packv[:, :, 0:1], sum_acc[:], axis=AX.X,
                                    op=ALU.add)
            nc.vector.tensor_reduce(packv[:, :, 1:2], sq_acc[:], axis=AX.X,
                                    op=ALU.add)
            nc.sync.dma_start(st_in_d[:], pack[:])
            if use_collective:
                nc.gpsimd.collective_compute(
                    "AllReduce", ALU.add,
                    replica_groups=[list(range(n_devices))],
                    ins=[st_in_d[:]],
                    outs=[st_out_d[:]],
                )
            else:
                nc.sync.dma_start(st_out_d[:], st_in_d[:])
            red = statp.tile([P, 2 * OC], f32)
            nc.sync.dma_start(red[:], st_out_d[:])
            a_sb = statp.tile([P, OC], f32)
            b2_sb = statp.tile([P, OC], f32)
            inv_nb = 1.0 / float(B * N)
            redv = red[:].rearrange("p (o two) -> p o two", two=2)
            mean = statp.tile([P, OC], f32)
            nc.vector.tensor_scalar(mean[:], redv[:, :, 0], inv_nb, None,
                                    ALU.mult)
            var = statp.tile([P, OC], f32)
            nc.vector.tensor_scalar(var[:], redv[:, :, 1], inv_nb, None,
                                    ALU.mult)
            msq = statp.tile([P, OC], f32)
            nc.vector.tensor_tensor(msq[:], mean[:], mean[:], ALU.mult)
            nc.vector.tensor_tensor(var[:], var[:], msq[:], ALU.subtract)
            # rstd = exp(-0.5*ln(var+eps))
            nc.vector.tensor_scalar(var[:], var[:], BN_EPS, None, ALU.add)
            nc.scalar.activation(var[:], var[:], AF.Ln)
            nc.scalar.activation(var[:], var[:], AF.Exp, scale=-0.5)
            nc.vector.tensor_tensor(a_sb[:], gm_sb[:], var[:], ALU.mult)
            nc.vector.tensor_tensor(msq[:], mean[:], a_sb[:], ALU.mult)
            nc.vector.tensor_tensor(b2_sb[:], bt_sb[:], msq[:],
                                    ALU.subtract)

            if debug_dumps:
                nc.sync.dma_start(dbg["dbg_pack"], pack[:])
                abp = statp.tile([P, 2 * OC], f32, name="abp")
                nc.vector.tensor_copy(abp[:, :OC], a_sb[:])
                nc.vector.tensor_copy(abp[:, OC:], b2_sb[:])
                nc.sync.dma_start(dbg["dbg_ab"], abp[:])
            # ---- final: out = relu(h*a + b2 + x) ----
            fchunks = [(b, o2, fc) for b in range(BPC)
                       for o2 in range(OC) for fc in range(FCH)]

            def f_front(fi, b, o2, fc):
                hap = h_of[b][:, o2, fc * 1024:(fc + 1) * 1024]
                xap = xb_sb[b][:, o2, fc * 1024:(fc + 1) * 1024]
                t1 = fstage.tile([P, 1024], bf16, tag="t1")
                nc.vector.tensor_scalar(t1[:], hap, a_sb[:, o2:o2 + 1],
                                        b2_sb[:, o2:o2 + 1],
                                        ALU.mult, ALU.add)
                t2 = fstage.tile([P, 1024], bf16, tag="t2")
                if fi % 4 == 3:
                    nc.gpsimd.tensor_tensor(t2[:], t1[:], xap, ALU.add)
                else:
                    nc.vector.tensor_tensor(t2[:], t1[:], xap, ALU.add)
                return t2

            prev = f_front(0, *fchunks[0])
            for fi, (b, o2, fc) in enumerate(fchunks):
                t2 = prev
                if fi + 1 < len(fchunks):
                    prev = f_front(fi + 1, *fchunks[fi + 1])
                otile = fstage.tile([P, 1024], bf16, tag="ot")
                if fi % 4 == 0:
                    nc.vector.tensor_scalar(otile[:], t2[:],
                                            0.0, None, ALU.max)
                else:
                    nc.scalar.activation(otile[:], t2[:], AF.Relu)
                nc.sync.dma_start(
                    out_d[b, o2 * P:(o2 + 1) * P,
                          fc * 1024:(fc + 1) * 1024], otile[:])

    _hoist_extra_waits(nc)
    return nc


_ENGINE_SEM_PREFIX = {
    "EngineType.PE": "PE_",
    "EngineType.Activation": "Activation_",
    "EngineType.DVE": "DVE_",
    "EngineType.Pool": "Pool_",
    "EngineType.SP": "SP_",
}


def _hoist_extra_waits(nc):
    """This walrus build rejects compute-engine instructions carrying more
    than one sync wait. Engine queues are strict FIFO, so (a) an
    instruction waiting on its own engine's semaphore is always already
    satisfied -> drop it; (b) any extra waits can be hoisted onto NoOp
    instructions injected just before, one wait each -- identical
    semantics."""
    import concourse.mybir as mybir
    nid = 0
    for blk in nc.m.functions[0].blocks:
        out = []
        changed = False
        for i in blk.instructions:
            si = getattr(i, "sync_info", None)
            eng = str(getattr(i, "engine", None))
            waits = list(si.on_wait) if si and si.on_wait else []
            if len(waits) > 1 and eng in _ENGINE_SEM_PREFIX:
                selfp = _ENGINE_SEM_PREFIX[eng]
                waits = [w for w in waits if not w.ant_name.startswith(selfp)]
                for w in waits[:-1]:
                    nid += 1
                    out.append(mybir.InstNoOp(
                        name=f"I-waitnop-{nid}",
                        engine=i.engine,
                        sync_info=mybir.SyncInfo(on_wait=[w], on_update=[]),
                        bass_nofuse=True,
                    ))
                i.sync_info = mybir.SyncInfo(
                    on_wait=waits[-1:], on_update=list(si.on_update or []))
                changed = True
            out.append(i)
        if changed:
            blk.instructions = out


def get_nc():
    if "nc" not in _cache:
        _cache["nc"] = _build_nc()
    return _cache["nc"]


def run(inputs_by_core, trace=False):
    from concourse.bass_utils import run_bass_kernel_spmd
    nc = get_nc()
    return run_bass_kernel_spmd(nc, inputs_by_core, list(range(NCORES)),
                                trace=trace)


def make_in_maps(x, mu, stem_w, stem_b, head_w, head_b, bn_gamma, bn_beta):
    bf16 = np.float16

    x = np.ascontiguousarray(np.asarray(x, np.float32)).reshape(B, C, N)
    mu = np.asarray(mu, np.float32)
    stem_w = np.asarray(stem_w, np.float32)
    stem_b = np.asarray(stem_b, np.float32)
    head_w = np.asarray(head_w, np.float32)

    def tile_rows(m):  # (C, F) -> (P, OC, F) with row t*P+p -> [p, t]
        return np.ascontiguousarray(
            m.reshape(OC, P, -1).transpose(1, 0, 2).astype(bf16))

    mut0 = stem_w.T @ mu                    # (C, K)
    mub0 = (mu.T @ stem_b)[None, :]         # (1, K)

    common = {
        "ws": tile_rows(stem_w),
        "wst": tile_rows(stem_w.T),
        "hwt": tile_rows(head_w.T),
        "mut0": tile_rows(mut0),
        "mub0": np.ascontiguousarray(mub0.astype(bf16)),
        "bsr": np.ascontiguousarray(stem_b[None, :].astype(bf16)),
        "bsc": np.ascontiguousarray(
            stem_b.reshape(OC, P).T.astype(bf16)),
        "onec": np.ones((P, 1), bf16),
        "oner": np.ones((1, P), bf16),
        "gm": np.ascontiguousarray(
            np.asarray(bn_gamma, np.float32).reshape(OC, P).T),
        "bt": np.ascontiguousarray(
            np.asarray(bn_beta, np.float32).reshape(OC, P).T),
    }
    maps = []
    for i in range(NCORES):
        xc = x[i * BPC:(i + 1) * BPC]                     # (BPC, C, N)
        xbt = xc.reshape(BPC, OC, P, N).transpose(0, 2, 1, 3)
        xtt = (xc.transpose(0, 2, 1)                      # (BPC, N, C)
               .reshape(BPC, NT, P, C).transpose(0, 2, 1, 3))
        maps.append({
            "xb": np.ascontiguousarray(xbt.astype(bf16)),
            "xt": np.ascontiguousarray(xtt.astype(bf16)),
            **common,
        })
    return maps


def kernel(x, mu, stem_w, stem_b, head_w, head_b, bn_gamma, bn_beta):
    in_maps = make_in_maps(x, mu, stem_w, stem_b, head_w, head_b,
                           bn_gamma, bn_beta)
    res = run(in_maps, trace=False)
    out = np.concatenate([res.results[i]["out"] for i in range(NCORES)],
                         axis=0)
    return out.reshape(B, C, 64, 64).astype(np.float32)
